# revision 1
# baseline (speedup 1.0000x reference)
"""Trainium2 Bass kernel for causal attention with additive bias + key padding mask.

Problem: B=2, H=16, S=2048, D=128 (fp32), attn_bias [H,S,S], mask [B,1,1,S], offset=0.

Sharding: 32 (b,h) pairs across 8 cores, mixed-batch: core c gets heads (2c, 2c+1)
of BOTH batches so every core sees the same mix of key-length caps.

Device math (per core, scores kept transposed: s_T[j, q], so no transposes):
  s_T = KT_blk^T @ QT_chunk          (PE, bf16, psum f32)
  pt  = exp(s_T)                     (ACT, psum -> sbuf fp16, 3-block grouped)
  ptm = pt * ebias                   (DVE fp16 2x; ebias = exp(bias) with causal +
                                      key-pad masks folded as exact zeros, fp16,
                                      packed ragged and fully SBUF-resident)
  out_T[d,q] += V_blk^T @ ptm        (PE, fp16 rhs)
  sums hybrid: leading fs blocks per chunk via PE one-hot-ones matmul into a
        per-head [NQC,512] psum (one DVE drain per head); remaining blocks
        accumulated into a fp16 sacc tile on DVE (copy-init). Host combines.
The o_ps drain (psum -> bf16 staging) alternates between the ACT and DVE
engines per chunk to balance queue load; the B-stage trails the A-stage by 4
groups globally so the pipeline never drains at chunk/head boundaries.
Final normalization (outT / sums) and transpose on host.
"""

import os
from contextlib import ExitStack

import ml_dtypes
import numpy as np

_B, _H, _S, _D = 2, 16, 2048, 128
_NCORES = 8
_NH = (_B * _H) // _NCORES  # heads per core = 4
_QCH = 512
_NQC = _S // _QCH
_G = 3  # blocks per exp/mult group
_FSX = 0.25  # fraction of each chunk's key blocks whose sums go via PE
_PSX = 0  # (unused) Pool sums share
_PIPE = 4  # group-level software pipeline lag for PV/sums emission

_PROG_CACHE = {}
LAST_RESULTS = None
LAST_IN_MAPS = None
LAST_BUILD_KW = None


def _schedule(caps, G=_G, fsx=_FSX, psx=_PSX):
    """Static per-core schedule; shared by host packing and device build."""
    plan = []
    off = 0
    for h, cap in enumerate(caps):
        hplan = []
        for qc in range(_NQC):
            q_end = (qc + 1) * _QCH
            jb_hi = min((q_end + 127) // 128, cap)
            groups = []
            g0 = 0
            while g0 < jb_hi:
                Gp = min(G, jb_hi - g0)
                qo = max(0, g0 * 128 - qc * _QCH)
                nq = _QCH - qo
                groups.append({"g0": g0, "Gp": Gp, "qo": qo, "nq": nq, "off": off})
                off += Gp * nq
                g0 += Gp
            fs = max(1, int(round(fsx * jb_hi))) if fsx > 0 else 0
            hplan.append({"jb_hi": jb_hi, "fs": fs, "groups": groups})
        plan.append(hplan)
    return plan, off


def _build_program(caps, repeat=1, G=_G, fsx=_FSX, psx=_PSX, pipe=_PIPE, drop="", unroll=False, gsum=0):
    import contextlib

    import concourse.bacc as bacc
    import concourse.mybir as mybir
    import concourse.tile as tile

    caps = tuple(caps)
    NH, S, D, QCH, NQC = _NH, _S, _D, _QCH, _NQC
    NB = S // 128
    kof = [sum(caps[:h]) for h in range(NH + 1)]  # ragged key-block offsets
    f32 = mybir.dt.float32
    f16 = mybir.dt.float16
    bf16 = mybir.dt.bfloat16

    plan, CF = _schedule(caps, G, fsx, psx)
    use_r = any(ch["fs"] > 0 for hp in plan for ch in hp)

    nc = bacc.Bacc("TRN2", target_bir_lowering=False, debug=False)

    KBT = kof[NH]  # total key blocks across heads
    kt_d = nc.dram_tensor("kt", [128, KBT * 128], bf16, kind="ExternalInput").ap()
    qt_d = nc.dram_tensor("qt", [NH, 128, S], bf16, kind="ExternalInput").ap()
    v_d = nc.dram_tensor("v", [128, KBT, D], f16, kind="ExternalInput").ap()
    eb_d = nc.dram_tensor("eb", [128, CF], f16, kind="ExternalInput").ap()
    outT_d = nc.dram_tensor("outT", [NH, D, S], bf16, kind="ExternalOutput").ap()
    SW = G if gsum else 1  # sacc slot width
    sacc_d = nc.dram_tensor(
        "sacc", [NH, NQC, 128, SW, QCH], f16, kind="ExternalOutput"
    ).ap()
    if use_r:
        r_d = nc.dram_tensor("r", [NH, NQC, QCH], f32, kind="ExternalOutput").ap()
        oh_d = nc.dram_tensor("oh", [128, NQC * NQC], f16, kind="ExternalInput").ap()

    with tile.TileContext(nc) as tc, ExitStack() as ctx:
        const = ctx.enter_context(tc.tile_pool(name="const", bufs=1))
        ptp = ctx.enter_context(tc.tile_pool(name="ptp", bufs=max(2, pipe - 1)))
        ptmp = ctx.enter_context(tc.tile_pool(name="ptmp", bufs=pipe + 1 + (1 if "ptm6" in drop else 0)))
        saccp = ctx.enter_context(tc.tile_pool(name="saccp", bufs=2))
        obp = ctx.enter_context(tc.tile_pool(name="obp", bufs=1 if pipe >= 5 else 2))
        rbp = ctx.enter_context(tc.tile_pool(name="rbp", bufs=1))
        psum_s = ctx.enter_context(tc.tile_pool(name="psum_s", bufs=3 if G == 2 else 2, space="PSUM"))
        psum_o = ctx.enter_context(tc.tile_pool(name="psum_o", bufs=1, space="PSUM"))
        if use_r:
            psum_r = ctx.enter_context(tc.tile_pool(name="psum_r", bufs=1, space="PSUM"))

        # one-hot "ones" weights (host-provided): oh[:, qc*NQC+qc] = 1, so chunk
        # qc's sums land in psum partition row qc of a per-head [NQC, QCH] accum
        if use_r:
            oh_sb = const.tile([128, NQC * NQC], f16)
            nc.sync.dma_start(out=oh_sb[:], in_=oh_d[:])
            ohs = [oh_sb[:, qc * NQC : (qc + 1) * NQC] for qc in range(NQC)]

        kt_sb = const.tile([128, KBT * 128], bf16)
        qt_sb = const.tile([128, NH, S], bf16)
        v_sb = const.tile([128, KBT, D], f16)
        eb_sb = const.tile([128, CF], f16)
        nc.sync.dma_start(out=kt_sb[:], in_=kt_d[:])
        nc.sync.dma_start(out=v_sb[:], in_=v_d[:])
        for h in range(NH):
            nc.sync.dma_start(out=qt_sb[:, h, :], in_=qt_d[h])
        nch = 8
        step = (CF + nch - 1) // nch
        for i in range(nch):
            lo = i * step
            hi = min(CF, lo + step)
            if lo < hi:
                (nc.sync if i % 2 else nc.gpsimd).dma_start(
                    out=eb_sb[:, lo:hi], in_=eb_d[:, lo:hi]
                )

        loop_cm = (
            tc.For_i(0, repeat, 1)
            if (repeat > 1 and not unroll)
            else contextlib.nullcontext()
        )
        with loop_cm:
          for _rep in range(repeat if unroll else 1):
            # flat stage list: one entry per (head, chunk, group); the B-stage
            # (PV + sums) trails the A-stage (QK + exp + mult) by `pipe` slots
            # globally, so the cross-engine pipeline never drains at chunk or
            # head boundaries.
            stages = []
            head_ctx = {}
            chunk_ctx = {}
            for h in range(NH):
                head_ctx[h] = {"r_ps": None}
                for qc in range(NQC):
                    chunk = plan[h][qc]
                    ck = {
                        "h": h,
                        "qc": qc,
                        "jb_hi": chunk["jb_hi"],
                        "fs": chunk["fs"],
                        "o_ps": None,
                        "sacc": None,
                    }
                    chunk_ctx[(h, qc)] = ck
                    ngrp = len(chunk["groups"])
                    for idx, g in enumerate(chunk["groups"]):
                        stages.append((ck, g, idx == ngrp - 1))

            stash = {}
            for gi in range(len(stages) + pipe):
                if gi < len(stages):
                    ck, g, _last = stages[gi]
                    h, qc = ck["h"], ck["qc"]
                    g0, Gp, qo, nq, off = (
                        g["g0"], g["Gp"], g["qo"], g["nq"], g["off"],
                    )
                    s3 = psum_s.tile([128, G, QCH], f32)
                    for i in range(Gp):
                        jb = g0 + i
                        nc.tensor.matmul(
                            s3[:, i, qo:],
                            lhsT=kt_sb[
                                :, (kof[h] + jb) * 128 : (kof[h] + jb + 1) * 128
                            ],
                            rhs=qt_sb[:, h, qc * QCH + qo : (qc + 1) * QCH],
                            start=True,
                            stop=True,
                        )
                    pt3 = ptp.tile([128, G, QCH], f16)
                    nc.scalar.activation(
                        pt3[:, :Gp, qo:],
                        s3[:, :Gp, qo:],
                        mybir.ActivationFunctionType.Exp,
                    )
                    ptm3 = ptmp.tile([128, G, QCH], f16)
                    ebv = eb_sb[:, off : off + Gp * nq].rearrange(
                        "p (g n) -> p g n", g=Gp
                    )
                    if "mult" not in drop:
                        nc.vector.tensor_mul(ptm3[:, :Gp, qo:], pt3[:, :Gp, qo:], ebv)
                    else:
                        ptm3 = pt3
                    stash[gi] = ptm3
                bi = gi - pipe
                if 0 <= bi < len(stages):
                    ck, g, last = stages[bi]
                    h, qc = ck["h"], ck["qc"]
                    jb_hi, fs = ck["jb_hi"], ck["fs"]
                    g0, Gp, qo = g["g0"], g["Gp"], g["qo"]
                    ptm3 = stash.pop(bi)
                    if ck["o_ps"] is None:
                        ck["o_ps"] = psum_o.tile([128, QCH], f32, name="o_ps")
                        if fs < jb_hi:
                            ck["sacc"] = saccp.tile([128, SW, QCH], f16, name="sacc")
                            if gsum:
                                nc.vector.memset(ck["sacc"][:], 0.0)
                    if fs > 0 and head_ctx[h]["r_ps"] is None:
                        head_ctx[h]["r_ps"] = psum_r.tile(
                            [NQC, QCH], f32, name="r_ps"
                        )
                    o_ps, sacc = ck["o_ps"], ck["sacc"]
                    r_ps = head_ctx[h]["r_ps"]
                    for i in range(Gp):
                        jb = g0 + i
                        qb = max(qo, jb * 128 - qc * QCH)  # per-block trim
                        if "pv" not in drop:
                            nc.tensor.matmul(
                                o_ps[:, qb:],
                                lhsT=v_sb[:, kof[h] + jb, :],
                                rhs=ptm3[:, i, qb:],
                                start=(jb == 0),
                                stop=(jb == jb_hi - 1),
                            )
                        elif jb == 0:
                            nc.tensor.matmul(
                                o_ps[:, :],
                                lhsT=v_sb[:, kof[h] + jb, :],
                                rhs=ptm3[:, i, :],
                                start=True,
                                stop=True,
                            )
                        if jb < fs:
                            nc.tensor.matmul(
                                r_ps[:, qb:],
                                lhsT=ohs[qc],
                                rhs=ptm3[:, i, qb:],
                                start=(qc == 0 and jb == 0),
                                stop=(qc == NQC - 1 and jb == fs - 1),
                                skip_group_check=True,
                            )
                        elif "sacc" in drop:
                            pass
                        elif gsum:
                            if g0 < fs:  # straddle group: per-block add
                                nc.vector.tensor_add(
                                    sacc[:, i, qb:], sacc[:, i, qb:], ptm3[:, i, qb:]
                                )
                        elif jb == fs:
                            nc.vector.tensor_copy(sacc[:, 0, qb:], ptm3[:, i, qb:])
                            if qb > 0:
                                nc.gpsimd.memset(sacc[:, 0, :qb], 0.0)
                        else:
                            nc.vector.tensor_add(
                                sacc[:, 0, qb:], sacc[:, 0, qb:], ptm3[:, i, qb:]
                            )
                    if gsum and g0 >= fs and "sacc" not in drop:
                        # whole group accumulated in one DVE op (slot per block)
                        nc.vector.tensor_add(
                            sacc[:, :Gp, qo:], sacc[:, :Gp, qo:], ptm3[:, :Gp, qo:]
                        )
                    if last:
                        ob = obp.tile([128, QCH], bf16)
                        if "obdve" in drop:
                            nc.vector.tensor_copy(ob[:], o_ps[:])
                        elif "obact" not in drop and (h * NQC + qc) % 2 == 0:
                            nc.vector.tensor_copy(ob[:], o_ps[:])
                        else:
                            nc.scalar.copy(ob[:], o_ps[:])
                        nc.gpsimd.dma_start(
                            out=outT_d[h, :, qc * QCH : (qc + 1) * QCH], in_=ob[:]
                        )
                        if sacc is not None and "sacc" not in drop:
                            nc.gpsimd.dma_start(out=sacc_d[h, qc], in_=sacc[:])
                        if fs > 0 and qc == NQC - 1:
                            rb_h = rbp.tile([NQC, QCH], f32)
                            nc.vector.tensor_copy(rb_h[:], r_ps[:])
                            nc.sync.dma_start(out=r_d[h], in_=rb_h[:])
                            head_ctx[h]["r_ps"] = None

    nc.compile()
    return nc


def _pack_ebias(eb_masked, caps, G=_G, fsx=_FSX, psx=_PSX):
    """eb_masked: [NH, S(j), S(q)] f32 (exp(bias) with masks folded as 0).
    Returns [128, CF] fp16 ragged-packed per the schedule."""
    plan, CF = _schedule(caps, G, fsx, psx)
    out = np.zeros((128, CF), dtype=np.float16)
    for h, hplan in enumerate(plan):
        for qc, chunk in enumerate(hplan):
            for g in chunk["groups"]:
                g0, Gp, qo, nq, off = g["g0"], g["Gp"], g["qo"], g["nq"], g["off"]
                for i in range(Gp):
                    jb = g0 + i
                    blk = eb_masked[
                        h,
                        jb * 128 : (jb + 1) * 128,
                        qc * _QCH + qo : (qc + 1) * _QCH,
                    ]
                    out[:, off + i * nq : off + (i + 1) * nq] = blk.astype(np.float16)
    return out


def _run_multicore(in_maps, caps):
    global LAST_RESULTS, LAST_IN_MAPS, LAST_BUILD_KW
    from concourse.bass_utils import run_bass_kernel_spmd

    key = (tuple(caps), _G, _FSX, _PSX, _PIPE)
    if key not in _PROG_CACHE:
        _PROG_CACHE[key] = _build_program(caps)
    nc = _PROG_CACHE[key]
    LAST_IN_MAPS = in_maps
    LAST_BUILD_KW = {"caps": tuple(caps), "G": _G, "fsx": _FSX, "psx": _PSX, "pipe": _PIPE}
    res = run_bass_kernel_spmd(nc, in_maps, core_ids=list(range(len(in_maps))))
    LAST_RESULTS = res
    return res.results


def kernel(q, k, v, mask, attn_bias, offset):
    B, H, S, D = _B, _H, _S, _D
    q = np.asarray(q, dtype=np.float32)
    k = np.asarray(k, dtype=np.float32)
    v = np.asarray(v, dtype=np.float32)
    mask = np.asarray(mask).astype(bool)
    attn_bias = np.asarray(attn_bias, dtype=np.float32)
    off = int(np.asarray(offset))

    scale = np.float32(D**-0.5)
    valid = mask[:, 0, 0, :]  # [B, S]

    caps_b = []
    for b in range(B):
        idx = np.nonzero(valid[b])[0]
        lv = (int(idx[-1]) + 1) if len(idx) else 1
        caps_b.append(max(1, (lv + 127) // 128))

    # ebias[h][j, i] = exp(attn_bias[h, i, j]), causal mask j >= i+1-off -> 0
    jj = np.arange(S)[:, None]
    ii = np.arange(S)[None, :]
    keep_causal = (jj < (ii + 1 - off)).astype(np.float32)  # [j, i]
    ebias = np.exp(attn_bias).transpose(0, 2, 1) * keep_causal[None]

    core_pairs = [
        [(0, 2 * c), (0, 2 * c + 1), (1, 2 * c), (1, 2 * c + 1)] for c in range(_NCORES)
    ]
    caps = tuple(caps_b[b] for (b, _h) in core_pairs[0])

    NB = S // 128
    bf = ml_dtypes.bfloat16
    in_maps = []
    for c in range(_NCORES):
        pairs = core_pairs[c]
        kt = np.concatenate(
            [(k[b, h][: caps_b[b] * 128] * scale).T for (b, h) in pairs], axis=1
        ).astype(bf)
        qt = np.stack([q[b, h].T for (b, h) in pairs]).astype(bf)
        vv = np.concatenate(
            [
                v[b, h][: caps_b[b] * 128].reshape(caps_b[b], 128, D).transpose(1, 0, 2)
                for (b, h) in pairs
            ],
            axis=1,
        ).astype(np.float16)
        ebm = np.stack(
            [ebias[h] * valid[b][:, None].astype(np.float32) for (b, h) in pairs]
        )
        oh = np.zeros((128, _NQC * _NQC), np.float16)
        for qc in range(_NQC):
            oh[:, qc * _NQC + qc] = 1.0
        in_maps.append(
            {
                "kt": np.ascontiguousarray(kt),
                "qt": np.ascontiguousarray(qt),
                "v": np.ascontiguousarray(vv),
                "eb": _pack_ebias(ebm, caps),
                "oh": oh,
            }
        )

    results = _run_multicore(in_maps, caps)

    out = np.empty((B, H, S, D), dtype=np.float32)
    for c in range(_NCORES):
        res = results[c]
        outT = res["outT"].astype(np.float32)  # [NH, D, S]
        sums = res["sacc"].astype(np.float32).sum(axis=(2, 3)).reshape(_NH, S)
        if "r" in res:
            sums = sums + res["r"].reshape(_NH, S)
        for i, (b, h) in enumerate(core_pairs[c]):
            out[b, h] = (outT[i] / sums[i][None, :]).T
    return out

